# revision 37
# baseline (speedup 1.0000x reference)
"""MoE gating kernel for Trainium2 (Bass/Tile), data-parallel over 8 NeuronCores.

Computes: logits = x @ W_g.T ; top-2 values; softmax over the 2 values.
  p1 = sigmoid(v1 - v2), p2 = sigmoid(v2 - v1)  (v1 >= v2 the top-2 logits)

Sharding: tokens split 8 ways (2048 tokens/core), W_g replicated.

Design (measured 71.8us vs 75.4us for the f32r/transpose-mode baseline):
  - x is cast fp32->bf16 during the HBM->SBUF DMA (SWDGE/gpsimd), one
    SIMPLE [128, 2048] DMA per token block: HBM reads (the ~45us
    roofline) are unchanged but the whole on-chip pipeline runs 16-bit.
    Simple 2D APs take SWDGE's packed descriptor path (~2.2us of Q7
    emission per tile); fancier rearranged APs cost ~3x the Q7 time and
    out-pace the SDMA stream.
  - W_g arrives pre-transposed/cast to bf16 from the host (weight layout
    prep) plus bf16/fp32 identities, via the sync/HWDGE ring (~+4us).
  - transposes are REGULAR bf16 matmuls against the identity moving
    operand (out = x_block.T @ I), ~56ns each warm via FWL: regular-MM
    activity keeps the HAM clock gate at 2.4GHz, while transpose-mode
    passes are invisible to HAM and earlier versions lost 20-40us at
    half clock to its re-throttling. Transpose output is fp32 in PSUM
    (TRN2 rule), so drains split 3:5 between DVE and the otherwise-idle
    ACT engine, casting to bf16 on the way out.
  - N=512 warm-up matmuls ALTERNATING two PSUM banks flip HAM ~3.4us
    after PE boot (single-bank warm-ups serialize on the WAW and their
    ~50% duty never flips it; N<=256 never flips it either); keeper
    matmuls at group boundaries split the PE's data-wait idle below
    HAM's warm MID window (1.7us).
  - sigmoids batch into one ACT call at the end (no Copy<->Sigmoid table
    switches mid-kernel) and the output leaves partition-major [128, 32]
    in ONE contiguous DMA (a token-major store is 2048 8-byte descriptors
    whose HBM write receipts add ~6us); the host de-interleaves.
bf16 adds ~4e-3 relative error on the output probabilities (gate is 2e-2).
"""

import sys

sys.path.insert(0, "/opt/trn_rl_repo")

from contextlib import ExitStack

import numpy as np
import ml_dtypes

import concourse.bass as bass
import concourse.bacc as bacc
import concourse.mybir as mybir
from concourse.tile import TileContext
from concourse.bass_utils import run_bass_kernel_spmd

TOKENS = 16384
DIM = 2048
E = 64  # num experts
NCORES = 8
TPC = TOKENS // NCORES  # tokens per core
P = 128
KT = DIM // P  # 16 contraction tiles
G = 256  # token group (moving-dim of the big matmul)
NG = TPC // G  # 8 groups per core
TB = G // P  # 2 token blocks per group
NB = NG * TB  # 16 token blocks per core

F32 = mybir.dt.float32
F32R = mybir.dt.float32r
BF16 = mybir.dt.bfloat16
N_WARM = 20  # bank-alternating N=512 warm-ups: ~80% PE duty (HAM needs that
# to flip) from PE boot until the first x tiles land (~+13.5us) -- a gap
# between warm-up end and first data re-throttles HAM within ~2us


def _emit(tc, ctx, x_ap, wgt_ap, idb_ap, idf_ap, out_ap):
    nc = tc.nc

    singles = ctx.enter_context(tc.tile_pool(name="singles", bufs=1))
    xpool = ctx.enter_context(tc.tile_pool(name="xpool", bufs=1))
    xtpool = ctx.enter_context(tc.tile_pool(name="xtpool", bufs=3))
    ltpool = ctx.enter_context(tc.tile_pool(name="ltpool", bufs=2))
    spool = ctx.enter_context(tc.tile_pool(name="spool", bufs=4))
    psum_t = ctx.enter_context(tc.tile_pool(name="psum_t", bufs=4, space="PSUM"))
    psum_l = ctx.enter_context(tc.tile_pool(name="psum_l", bufs=2, space="PSUM"))
    psum_f = ctx.enter_context(tc.tile_pool(name="psum_f", bufs=1, space="PSUM"))
    psum_w = ctx.enter_context(tc.tile_pool(name="psum_w", bufs=1, space="PSUM"))

    warm = singles.tile([P, P], BF16)
    warm_rhs = singles.tile([P, 4 * P], BF16)

    warm_flip = [False]

    def warm_mm():
        # alternate PSUM banks: back-to-back matmuls into ONE bank
        # serialize on the write-after-write and their ~50% duty never
        # flips HAM; alternating two banks sustains ~80% duty
        warm_flip[0] = not warm_flip[0]
        if warm_flip[0]:
            pw = psum_w.tile([P, 4 * P], F32, tag="warm_ps")
        else:
            pw = psum_f.tile([P, 4 * P], F32, tag="fin_ps")
        nc.tensor.matmul(pw[:], warm[:], warm_rhs[:])

    for _ in range(N_WARM):
        warm_mm()

    def keeper(n=1):
        # countable PE work with no dependencies: splits the PE's
        # data-wait idle below HAM's warm MID window; ~266ns each
        for _ in range(n):
            warm_mm()

    # constants on the sync ring, then the LAST group's two tiles as fp32
    # via the same (otherwise idle) HWDGE ring: they are resident by
    # ~+12us, so the final group is never gated by the SWDGE stream's
    # slow-engine stragglers
    ident = singles.tile([P, P], BF16)
    nc.sync.dma_start(out=ident[:], in_=idb_ap)
    ident_f = singles.tile([P, P], F32)
    nc.sync.dma_start(out=ident_f[:], in_=idf_ap)
    wgT = singles.tile([P, KT, E], BF16)
    nc.sync.dma_start(out=wgT[:], in_=wgt_ap)
    px7 = []
    for tb in range(TB):
        t = (NG - 1) * TB + tb
        pxt = xpool.tile([P, DIM], F32, tag=f"x{t}f")
        px7.append(pxt)
        nc.sync.dma_start(out=pxt[:], in_=x_ap[t * P : (t + 1) * P, :])

    # x tiles via SWDGE (gpsimd), bf16 cast during DMA. One SIMPLE
    # [128, 2048] DMA per tile: a simple 2D AP takes SWDGE's packed
    # descriptor path (~2.2us of Q7 emission per tile); fancier
    # rearranged APs cost ~3x the Q7 time and out-pace the SDMA stream.
    all_x = []
    for t in range(NB - TB):
        xt_in = xpool.tile([P, DIM], BF16, tag=f"x{t}")
        all_x.append(xt_in)
        nc.gpsimd.dma_start(out=xt_in[:], in_=x_ap[t * P : (t + 1) * P, :])
        if t == 1:
            # warm-up scratch memsets sit AFTER the first two tile DMAs on
            # the gpsimd queue: the x stream doorbells ~1.5us earlier, and
            # the warm-ups still start well before the first data lands
            nc.gpsimd.memset(warm[:], 0.0)
            nc.gpsimd.memset(warm_rhs[:], 0.0)

    # per-token-block v1-v2 / v2-v1 accumulate here; one sigmoid + one
    # contiguous partition-major store at the end
    dd_all = singles.tile([P, NB, 2], F32)

    def epilogue(g, lp):
        # back to token-major + top-2 (runs one group late)
        lt = ltpool.tile([E, G], F32)
        for tb in range(TB):
            nc.vector.tensor_copy(
                lt[:, tb * P : (tb + 1) * P], lp[:, tb * P : (tb + 1) * P]
            )
            fp = psum_f.tile([P, E], F32, tag="fin_ps")
            nc.tensor.matmul(
                fp[:],
                lt[:, tb * P : (tb + 1) * P],
                ident_f[:E, :E],
                is_transpose=True,
            )
            max8 = spool.tile([P, 8], F32)
            nc.vector.max(out=max8[:], in_=fp[:])
            b = g * TB + tb
            nc.vector.tensor_sub(dd_all[:, b, 0:1], max8[:, 0:1], max8[:, 1:2])
            nc.vector.tensor_sub(dd_all[:, b, 1:2], max8[:, 1:2], max8[:, 0:1])

    # the last group's transposes run fp32 transpose-mode from the
    # HWDGE-resident tiles, two k-pair phases at each late group boundary
    # (real work in place of keeper filler); by g6's chain, xt7 is done
    xt7 = singles.tile([P, KT * G], BF16)

    def g7_phase(q):
        pt = psum_t.tile([P, 2 * G], F32)
        for dk in range(2):
            k = 2 * q + dk
            for tb in range(TB):
                nc.tensor.matmul(
                    pt[:, dk * G + tb * P : dk * G + (tb + 1) * P],
                    px7[tb][:, k * P : (k + 1) * P],
                    ident_f[:],
                    is_transpose=True,
                )
        dst = xt7[:, 2 * q * G : (2 * q + 2) * G]
        if q < 3:
            nc.vector.tensor_copy(dst, pt[:])
        else:
            nc.scalar.copy(dst, pt[:])

    pending = None  # (g, lp) awaiting epilogue
    for g in range(NG - 1):
        xtiles = all_x[g * TB : (g + 1) * TB]

        if g >= 1:
            keeper(1)
        if g >= 3:
            # fp32 transpose-mode is invisible to HAM, so bracket the two
            # phases with a countable keeper
            g7_phase(2 * (g - 3))
            g7_phase(2 * (g - 3) + 1)
            keeper(1)

        # transpose into xT [128 d, k * G t] via REGULAR bf16 matmuls
        # against the identity (out = x_block.T @ I): counts as HAM
        # activity (transpose-mode would not) and sustains ~56ns each via
        # FWL. fp32 in PSUM (TRN2 rule), bf16 in SBUF. Each 2KB PSUM bank
        # holds 2 k-slices x 2 token blocks; drains split 3:5 DVE/ACT.
        xt = xtpool.tile([P, KT * G], BF16)
        for q in range(KT // 2):
            pt = psum_t.tile([P, 2 * G], F32)
            for dk in range(2):
                k = 2 * q + dk
                for tb in range(TB):
                    nc.tensor.matmul(
                        pt[:, dk * G + tb * P : dk * G + (tb + 1) * P],
                        xtiles[tb][:, k * P : (k + 1) * P],
                        ident[:],
                    )
            dst = xt[:, 2 * q * G : (2 * q + 2) * G]
            if q < 3:
                nc.vector.tensor_copy(dst, pt[:])
            else:
                nc.scalar.copy(dst, pt[:])

        # logitsT [64 e, 256 t] = sum_k wgT_k.T @ xT_k  (bf16 -> fp32 PSUM)
        lp = psum_l.tile([E, G], F32, tag="lp")
        for k in range(KT):
            nc.tensor.matmul(
                lp[:],
                wgT[:, k, :],
                xt[:, k * G : (k + 1) * G],
                start=(k == 0),
                stop=(k == KT - 1),
            )

        if pending is not None:
            epilogue(*pending)
        pending = (g, lp)

    # final group: xt7 is already transposed, only the chain remains
    lp7 = psum_l.tile([E, G], F32, tag="lp", name="lp")
    for k in range(KT):
        nc.tensor.matmul(
            lp7[:],
            wgT[:, k, :],
            xt7[:, k * G : (k + 1) * G],
            start=(k == 0),
            stop=(k == KT - 1),
        )
    epilogue(*pending)
    epilogue(NG - 1, lp7)

    # single sigmoid + one contiguous partition-major store
    ot = singles.tile([P, NB, 2], F32)
    nc.scalar.activation(ot[:], dd_all[:], mybir.ActivationFunctionType.Sigmoid)
    nc.sync.dma_start(out=out_ap, in_=ot[:])


_NC_CACHE = {}


def _build():
    key = "nc"
    if key in _NC_CACHE:
        return _NC_CACHE[key]
    nc = bacc.Bacc(trn_type="TRN2")
    x = nc.dram_tensor("x", [TPC, DIM], F32, kind="ExternalInput")
    wgt = nc.dram_tensor("wgt", [P, KT * E], BF16, kind="ExternalInput")
    idb = nc.dram_tensor("idb", [P, P], BF16, kind="ExternalInput")
    idf = nc.dram_tensor("idf", [P, P], F32, kind="ExternalInput")
    out = nc.dram_tensor("out", [P, NB * 2], F32, kind="ExternalOutput")
    with TileContext(nc) as tc, ExitStack() as ctx:
        _emit(tc, ctx, x.ap(), wgt.ap(), idb.ap(), idf.ap(), out.ap())
    if not nc.is_finalized():
        nc.finalize()
    _NC_CACHE[key] = nc
    return nc


def _run(x, W_g, trace=False):
    nc = _build()
    x = np.ascontiguousarray(np.asarray(x, dtype=np.float32))
    W_g = np.asarray(W_g, dtype=np.float32)
    # host-side weight layout prep: wgt[p, k*E + e] = W_g[e, k*128 + p]
    wgt = np.ascontiguousarray(
        W_g.reshape(E, KT, P).transpose(2, 1, 0).reshape(P, KT * E)
    ).astype(ml_dtypes.bfloat16)
    idb = np.eye(P, dtype=np.float32).astype(ml_dtypes.bfloat16)
    idf = np.eye(P, dtype=np.float32)
    in_maps = [
        {
            "x": np.ascontiguousarray(x[c * TPC : (c + 1) * TPC]),
            "wgt": wgt,
            "idb": idb,
            "idf": idf,
        }
        for c in range(NCORES)
    ]
    res = run_bass_kernel_spmd(nc, in_maps, core_ids=list(range(NCORES)), trace=trace)
    # device output is partition-major [128, 16, 2]; de-interleave:
    # out[b*128 + p, :] = res[p, b, :]
    outs = []
    for r in res.results:
        o = r["out"].reshape(P, NB, 2).transpose(1, 0, 2).reshape(TPC, 2)
        outs.append(o)
    out = np.ascontiguousarray(np.concatenate(outs, axis=0))
    return out, res


def kernel(x, W_g):
    out, _ = _run(x, W_g, trace=False)
    return out


def kernel_profiled(x, W_g, **_kw):
    out, res = _run(x, W_g, trace=True)
    return out, res


# revision 38
# speedup vs baseline: 1.0416x; 1.0416x over previous
"""MoE gating kernel for Trainium2 (Bass/Tile), data-parallel over 8 NeuronCores.

Computes: logits = x @ W_g.T ; top-2 values; softmax over the 2 values.
  p1 = sigmoid(v1 - v2), p2 = sigmoid(v2 - v1)  (v1 >= v2 the top-2 logits)

Sharding: tokens split 8 ways (2048 tokens/core), W_g replicated.

Design (measured 71.8us vs 75.4us for the f32r/transpose-mode baseline):
  - x is cast fp32->bf16 during the HBM->SBUF DMA (SWDGE/gpsimd), one
    SIMPLE [128, 2048] DMA per token block: HBM reads (the ~45us
    roofline) are unchanged but the whole on-chip pipeline runs 16-bit.
    Simple 2D APs take SWDGE's packed descriptor path (~2.2us of Q7
    emission per tile); fancier rearranged APs cost ~3x the Q7 time and
    out-pace the SDMA stream.
  - W_g arrives pre-transposed/cast to bf16 from the host (weight layout
    prep) plus bf16/fp32 identities, via the sync/HWDGE ring (~+4us).
  - transposes are REGULAR bf16 matmuls against the identity moving
    operand (out = x_block.T @ I), ~56ns each warm via FWL: regular-MM
    activity keeps the HAM clock gate at 2.4GHz, while transpose-mode
    passes are invisible to HAM and earlier versions lost 20-40us at
    half clock to its re-throttling. Transpose output is fp32 in PSUM
    (TRN2 rule), so drains split 3:5 between DVE and the otherwise-idle
    ACT engine, casting to bf16 on the way out.
  - N=512 warm-up matmuls ALTERNATING two PSUM banks flip HAM ~3.4us
    after PE boot (single-bank warm-ups serialize on the WAW and their
    ~50% duty never flips it; N<=256 never flips it either); keeper
    matmuls at group boundaries split the PE's data-wait idle below
    HAM's warm MID window (1.7us).
  - sigmoids batch into one ACT call at the end (no Copy<->Sigmoid table
    switches mid-kernel) and the output leaves partition-major [128, 32]
    in ONE contiguous DMA (a token-major store is 2048 8-byte descriptors
    whose HBM write receipts add ~6us); the host de-interleaves.
bf16 adds ~4e-3 relative error on the output probabilities (gate is 2e-2).
"""

import sys

sys.path.insert(0, "/opt/trn_rl_repo")

from contextlib import ExitStack

import numpy as np
import ml_dtypes

import concourse.bass as bass
import concourse.bacc as bacc
import concourse.mybir as mybir
from concourse.tile import TileContext
from concourse.bass_utils import run_bass_kernel_spmd

TOKENS = 16384
DIM = 2048
E = 64  # num experts
NCORES = 8
TPC = TOKENS // NCORES  # tokens per core
P = 128
KT = DIM // P  # 16 contraction tiles
G = 256  # token group (moving-dim of the big matmul)
NG = TPC // G  # 8 groups per core
TB = G // P  # 2 token blocks per group
NB = NG * TB  # 16 token blocks per core

F32 = mybir.dt.float32
F32R = mybir.dt.float32r
BF16 = mybir.dt.bfloat16
N_WARM = 16  # bank-alternating N=512 warm-ups: ~80% PE duty (HAM needs that to flip) from
# PE boot (~+2.5us) until the first x tiles land (~+8us)


def _emit(tc, ctx, x_ap, wgt_ap, idb_ap, idf_ap, out_ap):
    nc = tc.nc

    singles = ctx.enter_context(tc.tile_pool(name="singles", bufs=1))
    xpool = ctx.enter_context(tc.tile_pool(name="xpool", bufs=1))
    xtpool = ctx.enter_context(tc.tile_pool(name="xtpool", bufs=3))
    ltpool = ctx.enter_context(tc.tile_pool(name="ltpool", bufs=2))
    spool = ctx.enter_context(tc.tile_pool(name="spool", bufs=4))
    psum_t = ctx.enter_context(tc.tile_pool(name="psum_t", bufs=4, space="PSUM"))
    psum_l = ctx.enter_context(tc.tile_pool(name="psum_l", bufs=2, space="PSUM"))
    psum_f = ctx.enter_context(tc.tile_pool(name="psum_f", bufs=1, space="PSUM"))
    psum_w = ctx.enter_context(tc.tile_pool(name="psum_w", bufs=1, space="PSUM"))

    warm = singles.tile([P, P], BF16)
    warm_rhs = singles.tile([P, 4 * P], BF16)

    warm_flip = [False]

    def warm_mm():
        # alternate PSUM banks: back-to-back matmuls into ONE bank
        # serialize on the write-after-write and their ~50% duty never
        # flips HAM; alternating two banks sustains ~80% duty
        warm_flip[0] = not warm_flip[0]
        if warm_flip[0]:
            pw = psum_w.tile([P, 4 * P], F32, tag="warm_ps")
        else:
            pw = psum_f.tile([P, 4 * P], F32, tag="fin_ps")
        nc.tensor.matmul(pw[:], warm[:], warm_rhs[:])

    for _ in range(N_WARM):
        warm_mm()

    def keeper(n=1):
        # countable PE work with no dependencies: splits the PE's
        # data-wait idle below HAM's warm MID window; ~266ns each
        for _ in range(n):
            warm_mm()

    # constants on the sync ring
    ident = singles.tile([P, P], BF16)
    nc.sync.dma_start(out=ident[:], in_=idb_ap)
    ident_f = singles.tile([P, P], F32)
    nc.sync.dma_start(out=ident_f[:], in_=idf_ap)
    wgT = singles.tile([P, KT, E], BF16)
    nc.sync.dma_start(out=wgT[:], in_=wgt_ap)

    # x tiles via SWDGE (gpsimd), bf16 cast during DMA. One SIMPLE
    # [128, 2048] DMA per tile: a simple 2D AP takes SWDGE's packed
    # descriptor path (~2.2us of Q7 emission per tile); fancier
    # rearranged APs cost ~3x the Q7 time and out-pace the SDMA stream.
    all_x = []
    for t in range(NB):
        xt_in = xpool.tile([P, DIM], BF16, tag=f"x{t}")
        all_x.append(xt_in)
        nc.gpsimd.dma_start(out=xt_in[:], in_=x_ap[t * P : (t + 1) * P, :])
        if t == 1:
            # warm-up scratch memsets sit AFTER the first two tile DMAs on
            # the gpsimd queue: the x stream doorbells ~1.5us earlier, and
            # the warm-ups still start well before the first data lands
            nc.gpsimd.memset(warm[:], 0.0)
            nc.gpsimd.memset(warm_rhs[:], 0.0)

    # per-token-block v1-v2 / v2-v1 accumulate here; one sigmoid + one
    # contiguous partition-major store at the end
    dd_all = singles.tile([P, NB, 2], F32)

    def epilogue(g, lp):
        # back to token-major + top-2 (runs one group late)
        lt = ltpool.tile([E, G], F32)
        for tb in range(TB):
            nc.vector.tensor_copy(
                lt[:, tb * P : (tb + 1) * P], lp[:, tb * P : (tb + 1) * P]
            )
            fp = psum_f.tile([P, E], F32, tag="fin_ps")
            nc.tensor.matmul(
                fp[:],
                lt[:, tb * P : (tb + 1) * P],
                ident_f[:E, :E],
                is_transpose=True,
            )
            max8 = spool.tile([P, 8], F32)
            nc.vector.max(out=max8[:], in_=fp[:])
            b = g * TB + tb
            nc.vector.tensor_sub(dd_all[:, b, 0:1], max8[:, 0:1], max8[:, 1:2])
            nc.vector.tensor_sub(dd_all[:, b, 1:2], max8[:, 1:2], max8[:, 0:1])

    pending = None  # (g, lp) awaiting epilogue
    for g in range(NG):
        xtiles = all_x[g * TB : (g + 1) * TB]

        if g >= 1:
            # early groups: PE is still working off the start-up backlog,
            # so idle is short; late groups wait on arrivals (and SWDGE
            # engine-7/15 stragglers), so split that idle more finely
            keeper(1 if g < 4 else 2)

        # transpose into xT [128 d, k * G t] via REGULAR bf16 matmuls
        # against the identity (out = x_block.T @ I): counts as HAM
        # activity (transpose-mode would not) and sustains ~56ns each via
        # FWL. fp32 in PSUM (TRN2 rule), bf16 in SBUF. Each 2KB PSUM bank
        # holds 2 k-slices x 2 token blocks; drains split 3:5 DVE/ACT.
        xt = xtpool.tile([P, KT * G], BF16)
        for q in range(KT // 2):
            pt = psum_t.tile([P, 2 * G], F32)
            for dk in range(2):
                k = 2 * q + dk
                for tb in range(TB):
                    nc.tensor.matmul(
                        pt[:, dk * G + tb * P : dk * G + (tb + 1) * P],
                        xtiles[tb][:, k * P : (k + 1) * P],
                        ident[:],
                    )
            dst = xt[:, 2 * q * G : (2 * q + 2) * G]
            if q < 3:
                nc.vector.tensor_copy(dst, pt[:])
            else:
                nc.scalar.copy(dst, pt[:])

        # logitsT [64 e, 256 t] = sum_k wgT_k.T @ xT_k  (bf16 -> fp32 PSUM)
        lp = psum_l.tile([E, G], F32)
        for k in range(KT):
            nc.tensor.matmul(
                lp[:],
                wgT[:, k, :],
                xt[:, k * G : (k + 1) * G],
                start=(k == 0),
                stop=(k == KT - 1),
            )

        if pending is not None:
            epilogue(*pending)
        pending = (g, lp)
    epilogue(*pending)

    # single sigmoid + one contiguous partition-major store
    ot = singles.tile([P, NB, 2], F32)
    nc.scalar.activation(ot[:], dd_all[:], mybir.ActivationFunctionType.Sigmoid)
    nc.sync.dma_start(out=out_ap, in_=ot[:])


_NC_CACHE = {}


def _build():
    key = "nc"
    if key in _NC_CACHE:
        return _NC_CACHE[key]
    nc = bacc.Bacc(trn_type="TRN2")
    x = nc.dram_tensor("x", [TPC, DIM], F32, kind="ExternalInput")
    wgt = nc.dram_tensor("wgt", [P, KT * E], BF16, kind="ExternalInput")
    idb = nc.dram_tensor("idb", [P, P], BF16, kind="ExternalInput")
    idf = nc.dram_tensor("idf", [P, P], F32, kind="ExternalInput")
    out = nc.dram_tensor("out", [P, NB * 2], F32, kind="ExternalOutput")
    with TileContext(nc) as tc, ExitStack() as ctx:
        _emit(tc, ctx, x.ap(), wgt.ap(), idb.ap(), idf.ap(), out.ap())
    if not nc.is_finalized():
        nc.finalize()
    _NC_CACHE[key] = nc
    return nc


def _run(x, W_g, trace=False):
    nc = _build()
    x = np.ascontiguousarray(np.asarray(x, dtype=np.float32))
    W_g = np.asarray(W_g, dtype=np.float32)
    # host-side weight layout prep: wgt[p, k*E + e] = W_g[e, k*128 + p]
    wgt = np.ascontiguousarray(
        W_g.reshape(E, KT, P).transpose(2, 1, 0).reshape(P, KT * E)
    ).astype(ml_dtypes.bfloat16)
    idb = np.eye(P, dtype=np.float32).astype(ml_dtypes.bfloat16)
    idf = np.eye(P, dtype=np.float32)
    in_maps = [
        {
            "x": np.ascontiguousarray(x[c * TPC : (c + 1) * TPC]),
            "wgt": wgt,
            "idb": idb,
            "idf": idf,
        }
        for c in range(NCORES)
    ]
    res = run_bass_kernel_spmd(nc, in_maps, core_ids=list(range(NCORES)), trace=trace)
    # device output is partition-major [128, 16, 2]; de-interleave:
    # out[b*128 + p, :] = res[p, b, :]
    outs = []
    for r in res.results:
        o = r["out"].reshape(P, NB, 2).transpose(1, 0, 2).reshape(TPC, 2)
        outs.append(o)
    out = np.ascontiguousarray(np.concatenate(outs, axis=0))
    return out, res


def kernel(x, W_g):
    out, _ = _run(x, W_g, trace=False)
    return out


def kernel_profiled(x, W_g, **_kw):
    out, res = _run(x, W_g, trace=True)
    return out, res
